# revision 1
# baseline (speedup 1.0000x reference)
"""Brownian/OU bridge sampler kernel for Trainium2 (8 NeuronCores).

Problem (per batch element b, time series of length T, DIM=64 channels):
  first 32 channels:  bm = cumsum_t(sqrt(dt)*noise) / (sqrt(t)+1e-8)
  last 32 channels:   ou = e^{-theta t} * cumsum_t(sqrt(e^{2 theta t}-e^{2 theta t'})
                           * sigma/sqrt(2 theta) * noise)
                           / (sigma*sqrt((1-e^{-2 theta t})/(2 theta))+1e-8)

Strategy: pure data parallel over batch (32 samples per core); no cross-core
communication. Each 256-timestep chunk is loaded with a time-PAIR layout —
partition p holds t = 2p and 2p+1 — so every DMA descriptor covers 512 B
(the SDMA line-rate knee; a plain t-per-partition layout yields 256 B
descriptors and only ~134 GB/s vs ~300 GB/s measured for this pattern).
The 256-step cumsum is built from fp32 matmuls against a triangular-ones
stationary: S_odd = L^T r_even + L^T r_odd (+ carry), then
S_even = S_odd - r_odd on the vector engine. Cross-chunk carries ride a
third matmul whose stationary selects PSUM row 127 (the running total) out
of an aligned 32-row stash copied by the scalar engine. Per-timestep
coefficients are precomputed once on a compact [128, 512] layout (flat
full-speed ts load + PE-transpose redistribution) and broadcast into the
bulk passes with step-0 access patterns.

Numerics: exp(2θt)-exp(2θt') is restructured as exp(2θt')*expm1(2θ dt)
(Taylor expm1; dt<=1e-2 so a cubic is exact to 3e-10) and 1-exp(-2θt) uses
a degree-6 Taylor/direct blend at 2θt=0.5 — both avoid catastrophic
cancellation against the ACT engine's ~1e-5 exp error. Everything else is
fp32; the result matches a float64 pipeline to 1.15e-4, which is the
reference's own fp32 noise floor.
"""
import numpy as np

import bass_rust
import concourse.bass as bass
import concourse.tile as tile
from concourse import mybir
from concourse.bass_utils import run_bass_kernel_spmd

B, T, DIM = 256, 2048, 64
THETA = 0.1
N_CORES = 8
NB = B // N_CORES      # 32 samples per core
P = 128                # partitions
NC2 = T // (2 * P)     # 8 time chunks of 256 steps
S = 8                  # samples packed per matmul free dim
G = NB // S            # 4 carry chains per core
H = 2                  # halves (bm / ou)
DH = DIM // H          # 32
FREE = S * DIM         # 512 = one PSUM bank of fp32
L2 = 2                 # time-pair dimension

F32 = mybir.dt.float32
AF = mybir.ActivationFunctionType
OP = mybir.AluOpType


def _split_waits(nc, max_waits=1):
    """walrus in this container rejects >1 sem wait per instruction; hoist
    extras onto same-engine NoOps inserted just before the offender."""
    n = 0
    for f in nc.m.functions:
        for blk in f.blocks:
            insts = blk.instructions
            i = 0
            while i < len(insts):
                inst = insts[i]
                si = inst.sync_info
                if si is not None and len(si.on_wait) > max_waits:
                    waits = list(si.on_wait)
                    keep, rest = waits[:max_waits], waits[max_waits:]
                    nops = []
                    for j in range(0, len(rest), max_waits):
                        nop = bass_rust.InstNoOp(name=f"I-ws-{n}", ins=[], outs=[])
                        n += 1
                        nop.engine = inst.engine
                        nop.sync_info = mybir.SyncInfo(
                            on_wait=rest[j : j + max_waits], on_update=[])
                        nops.append(nop)
                    inst.sync_info = mybir.SyncInfo(
                        on_wait=keep, on_update=list(si.on_update))
                    for k, nop in enumerate(nops):
                        insts.insert(i + k, nop)
                    i += len(nops)
                i += 1
    return nc


def _strided(ap_full, offset_elems, step, count):
    """[P, count] view of a tile's free space at element offset with stride."""
    return bass.AP(
        tensor=ap_full.tensor,
        offset=ap_full.offset + offset_elems,
        ap=[list(ap_full.ap[0]), [step, count]],
    )


def _build(reps: int = 1):
    nc = bass.Bass("TRN2")
    ts_in = nc.dram_tensor("ts", [NB, T, 1], F32, kind="ExternalInput")
    nz_in = nc.dram_tensor("noise", [NB, T, DIM], F32, kind="ExternalInput")
    out = nc.dram_tensor("out", [NB, T, DIM], F32, kind="ExternalOutput")

    ts_flat = ts_in[:, :, 0].rearrange("s t -> (s t)")

    with tile.TileContext(nc) as tc:
        with (
            tc.tile_pool(name="consts", bufs=1) as consts,
            tc.tile_pool(name="cwork", bufs=1) as cwork,
            tc.tile_pool(name="nzp", bufs=6) as nzp,
            tc.tile_pool(name="rp", bufs=6) as rp,
            tc.tile_pool(name="tep", bufs=6) as tep,
            tc.tile_pool(name="op_", bufs=6) as op_,
            tc.tile_pool(name="psp", bufs=6, space="PSUM") as psp,
        ):
            # ---------------- constants ----------------
            ones_t = consts.tile([P, P], F32)
            nc.vector.memset(ones_t, 1.0)
            L = consts.tile([P, P], F32)          # L[u, q] = 1 if u <= q
            nc.gpsimd.affine_select(
                out=L, in_=ones_t, pattern=[[1, P]], compare_op=OP.is_ge,
                fill=0.0, base=0, channel_multiplier=-1)
            e31 = consts.tile([32, P], F32)       # row 31 ones, else 0
            nc.gpsimd.affine_select(
                out=e31, in_=ones_t[0:32, :], pattern=[[0, P]],
                compare_op=OP.is_equal, fill=0.0, base=-31,
                channel_multiplier=1)
            ident = consts.tile([P, P], F32)      # identity for PE transpose
            nc.gpsimd.affine_select(
                out=ident, in_=ones_t, pattern=[[-1, P]],
                compare_op=OP.is_equal, fill=0.0, base=0,
                channel_multiplier=1)

            # -------- compact per-timestep coefficients --------
            # Target layout: X[p, g, s', i2, l] holds t = i2*256 + 2p + l of
            # sample s = 8g + s'; flat free index = 2n + l with n = s*8 + i2.
            # Filled from a flat full-speed ts load via PE transposes with
            # stride-2 input APs: T_{hb,l}[p, q] = flat[q*512 + 256*hb + 2p + l]
            # lands at n = 2q + hb (free stride 4, offset 2*hb + l).
            s1 = consts.tile([P, 512], F32)       # flat[p*512 + f]
            nc.sync.dma_start(
                out=s1, in_=ts_flat.rearrange("(p f) -> p f", p=P))
            s1p = consts.tile([P, 512], F32)      # flat[p*512 + f - 1]
            nc.sync.dma_start(
                out=s1p[1:P, :],
                in_=bass.AP(tensor=ts_flat.tensor, offset=ts_flat.offset + 511,
                            ap=[[512, P - 1], [1, 512]]))
            nc.sync.dma_start(
                out=s1p[0:1, 1:512],
                in_=bass.AP(tensor=ts_flat.tensor, offset=ts_flat.offset,
                            ap=[[0, 1], [1, 511]]))
            nc.vector.memset(s1p[0:1, 0:1], 0.0)

            ts_c = consts.tile([P, G, S, NC2, L2], F32)
            tsp_c = consts.tile([P, G, S, NC2, L2], F32)
            tsf = ts_c[:, :, :, :, :].rearrange("p g s i l -> p (g s i l)")
            tspf = tsp_c[:, :, :, :, :].rearrange("p g s i l -> p (g s i l)")
            with tc.tile_pool(name="trps", bufs=2, space="PSUM") as trps:
                for src, dstf in ((s1, tsf), (s1p, tspf)):
                    for hb in range(2):
                        for lv in range(2):
                            pst = trps.tile([P, P], F32, tag="trp",
                                            name=f"trp{hb}{lv}")
                            nc.tensor.transpose(
                                out=pst,
                                in_=_strided(src[:, :], 256 * hb + lv, 2, P),
                                identity=ident)
                            nc.vector.tensor_copy(
                                out=_strided(dstf, 2 * hb + lv, 4, P),
                                in_=pst)
            # each sample's t=0 has predecessor time 0
            nc.vector.memset(tsp_c[0:1, :, :, 0:1, 0:1], 0.0)

            cmul = consts.tile([P, G, S, NC2, L2, H], F32)
            cnorm = consts.tile([P, G, S, NC2, L2, H], F32)
            cm0 = cmul[:, :, :, :, :, 0].rearrange("p g s i l -> p (g s i l)")
            cm1 = cmul[:, :, :, :, :, 1].rearrange("p g s i l -> p (g s i l)")
            cn0 = cnorm[:, :, :, :, :, 0].rearrange("p g s i l -> p (g s i l)")
            cn1 = cnorm[:, :, :, :, :, 1].rearrange("p g s i l -> p (g s i l)")

            NF = G * S * NC2 * L2  # 512
            t0 = cwork.tile([P, NF], F32, tag="t0")
            t1 = cwork.tile([P, NF], F32, tag="t1")
            t2 = cwork.tile([P, NF], F32, tag="t2")
            t3 = cwork.tile([P, NF], F32, tag="t3")
            t4 = cwork.tile([P, NF], F32, tag="t4")

            # db = sqrt(ts - tsp)   (fp32 subtraction is exact here)
            nc.vector.tensor_tensor(out=t0, in0=tsf, in1=tspf, op=OP.subtract)
            nc.scalar.activation(out=cm0, in_=t0, func=AF.Sqrt)
            # dou = sqrt(5 * exp(.2 tsp) * expm1(.2 (ts-tsp)))
            nc.vector.tensor_scalar_mul(out=t1, in0=t0, scalar1=0.2)     # x
            nc.vector.tensor_scalar(out=t2, in0=t1, scalar1=1.0 / 3.0,
                                    scalar2=1.0, op0=OP.mult, op1=OP.add)
            nc.vector.tensor_mul(out=t3, in0=t1, in1=t2)
            nc.vector.tensor_scalar(out=t2, in0=t3, scalar1=0.5,
                                    scalar2=1.0, op0=OP.mult, op1=OP.add)
            nc.vector.tensor_mul(out=t3, in0=t1, in1=t2)                 # expm1
            nc.scalar.activation(out=t2, in_=tspf, func=AF.Exp, scale=0.2)
            nc.vector.tensor_mul(out=t3, in0=t3, in1=t2)
            nc.scalar.activation(out=cm1, in_=t3, func=AF.Sqrt, scale=5.0)
            # nb = 1/(sqrt(ts)+1e-8)
            nc.scalar.activation(out=t0, in_=tsf, func=AF.Sqrt)
            nc.vector.tensor_scalar_add(out=t0, in0=t0, scalar1=1e-8)
            nc.vector.reciprocal(out=cn0, in_=t0)
            # f2 = exp(-.1 ts) / (sqrt(5*(1-exp(-.2 ts))) + 1e-8)
            #   1-exp(-y), y = .2 ts: Taylor (deg 6) below y=0.5 else direct
            nc.vector.tensor_scalar_mul(out=t0, in0=tsf, scalar1=0.2)    # y
            nc.scalar.activation(out=t1, in_=tsf, func=AF.Exp, scale=-0.2)
            nc.vector.tensor_scalar(out=t1, in0=t1, scalar1=-1.0,
                                    scalar2=1.0, op0=OP.mult, op1=OP.add)
            nc.vector.tensor_scalar(out=t2, in0=t0, scalar1=-1.0 / 6.0,
                                    scalar2=1.0, op0=OP.mult, op1=OP.add)
            for k in (5, 4, 3, 2):
                nc.vector.tensor_mul(out=t3, in0=t0, in1=t2)
                nc.vector.tensor_scalar(out=t2, in0=t3, scalar1=-1.0 / k,
                                        scalar2=1.0, op0=OP.mult, op1=OP.add)
            nc.vector.tensor_mul(out=t3, in0=t0, in1=t2)                 # taylor
            nc.vector.tensor_scalar(out=t4, in0=t0, scalar1=0.5, scalar2=None,
                                    op0=OP.is_lt)
            nc.vector.tensor_tensor(out=t3, in0=t3, in1=t1, op=OP.subtract)
            nc.vector.tensor_mul(out=t3, in0=t4, in1=t3)
            nc.vector.tensor_tensor(out=t3, in0=t3, in1=t1, op=OP.add)   # w2
            nc.scalar.activation(out=t3, in_=t3, func=AF.Sqrt, scale=5.0)
            nc.vector.tensor_scalar_add(out=t3, in0=t3, scalar1=1e-8)
            nc.vector.reciprocal(out=t3, in_=t3)
            nc.scalar.activation(out=t0, in_=tsf, func=AF.Exp, scale=-0.1)
            nc.vector.tensor_mul(out=cn1, in0=t0, in1=t3)

            # ---------------- main scan ----------------
            ctmp = [consts.tile([32, FREE], F32, tag=f"ctmp{g}", name=f"ctmp{g}")
                    for g in range(G)]
            for _rep in range(reps):
                for i2 in range(NC2):
                    for g in range(G):
                        s8 = g * S
                        tsl = slice(i2 * 2 * P, (i2 + 1) * 2 * P)
                        nt_t = nzp.tile([P, S, L2 * DIM], F32, tag="nz")
                        nc.sync.dma_start(
                            out=nt_t,
                            in_=nz_in[s8 : s8 + S, tsl, :].rearrange(
                                "s (p l) d -> p s (l d)", l=L2))
                        r = rp.tile([P, S, L2, DIM], F32, tag="r")
                        for lv in range(L2):
                            nc.vector.tensor_mul(
                                out=r[:, :, lv, :].rearrange(
                                    "p s (h d) -> p s h d", h=H),
                                in0=nt_t[:, :, lv * DIM : (lv + 1) * DIM]
                                .rearrange("p s (h d) -> p s h d", h=H),
                                in1=cmul[:, g, :, i2, lv, :].to_broadcast(
                                    [P, S, H, DH]))
                        ps = psp.tile([P, FREE], F32, tag="ps")
                        re = r[:, :, 0, :]
                        ro = r[:, :, 1, :]
                        nc.tensor.matmul(ps, lhsT=L, rhs=re,
                                         start=True, stop=False)
                        nc.tensor.matmul(ps, lhsT=L, rhs=ro,
                                         start=False, stop=(i2 == 0))
                        if i2 > 0:
                            nc.tensor.matmul(ps, lhsT=e31, rhs=ctmp[g],
                                             start=False, stop=True)
                        if i2 < NC2 - 1:
                            nc.scalar.activation(out=ctmp[g], in_=ps[96:128, :],
                                                 func=AF.Copy)
                        o = op_.tile([P, S, L2 * DIM], F32, tag="o")
                        # odd time positions: S_odd * cnorm
                        nc.vector.tensor_mul(
                            out=o[:, :, DIM : L2 * DIM].rearrange(
                                "p s (h d) -> p s h d", h=H),
                            in0=ps[:, :].rearrange(
                                "p (s h d) -> p s h d", s=S, h=H),
                            in1=cnorm[:, g, :, i2, 1, :].to_broadcast(
                                [P, S, H, DH]))
                        # even time positions: (S_odd - r_odd) * cnorm
                        te = tep.tile([P, S, DIM], F32, tag="te")
                        nc.vector.tensor_tensor(
                            out=te,
                            in0=ps[:, :].rearrange("p (s d) -> p s d", s=S),
                            in1=ro, op=OP.subtract)
                        nc.gpsimd.tensor_mul(
                            out=o[:, :, 0:DIM].rearrange(
                                "p s (h d) -> p s h d", h=H),
                            in0=te[:, :, :].rearrange(
                                "p s (h d) -> p s h d", h=H),
                            in1=cnorm[:, g, :, i2, 0, :].to_broadcast(
                                [P, S, H, DH]))
                        nc.sync.dma_start(
                            out=out[s8 : s8 + S, tsl, :].rearrange(
                                "s (p l) d -> p s (l d)", l=L2),
                            in_=o)
    _split_waits(nc)
    return nc


_NC = None


def _get_nc():
    global _NC
    if _NC is None:
        _NC = _build()
    return _NC


def kernel(ts: np.ndarray, noise: np.ndarray) -> np.ndarray:
    ts = np.ascontiguousarray(ts, dtype=np.float32)
    noise = np.ascontiguousarray(noise, dtype=np.float32)
    in_maps = [
        {"ts": ts[c * NB : (c + 1) * NB], "noise": noise[c * NB : (c + 1) * NB]}
        for c in range(N_CORES)
    ]
    res = run_bass_kernel_spmd(_get_nc(), in_maps, core_ids=list(range(N_CORES)))
    return np.concatenate([r["out"] for r in res.results], axis=0)



# revision 6
# speedup vs baseline: 1.3686x; 1.3686x over previous
"""Brownian/OU bridge sampler kernel for Trainium2 (8 NeuronCores), bf16 edition.

Problem (per batch element b, time series of length T, DIM=64 channels):
  first 32 channels:  bm = cumsum_t(sqrt(dt)*noise) / (sqrt(t)+1e-8)
  last 32 channels:   ou = e^{-theta t} * cumsum_t(sqrt(e^{2 theta t}-e^{2 theta t'})
                           * sigma/sqrt(2 theta) * noise)
                           / (sigma*sqrt((1-e^{-2 theta t})/(2 theta))+1e-8)

Strategy: pure data parallel over batch (32 samples per core). The rel-err
gate (2e-2) leaves room for bf16 data movement, which halves HBM traffic vs
the f32 pipeline: noise is staged to the device as bf16 and the output is
returned as bf16 (widened to f32 on the host). To keep 512-byte DMA
descriptors (the SDMA line-rate knee) at 2 bytes/elem, each 512-timestep
chunk uses a time-QUAD layout: partition q holds t = 4q+lv, lv=0..3. The
512-step cumsum is 4 accumulating bf16 matmuls against a triangular-ones
stationary (S3 = sum of all r_lv up to partition q), plus an f32 carry
matmul that broadcasts PSUM row 127 of the previous chunk from a 32-row
stash. S2/S1/S0 descend from S3 by subtracting r3/r2/r1 (spread across
Pool and DVE); a single fused DVE multiply applies the normalization.

Per-timestep coefficients are computed once in f32 on a compact [128, 512]
layout (flat full-speed ts load + 8 PE-transpose redistributions with
stride-4 access patterns), then written as PAIR-DUPLICATED bf16 tables
[q, n, 2] holding (c, c). Broadcasting a coefficient across the 32
channels of a half is done with access patterns whose innermost dim is the
packed pair (stride 1, count 2) and whose channel dim is stride 0 — every
operand then presents a packed 2-byte innermost axis, which keeps the DVE
in its 2x half-word mode (a stride-0 innermost axis would drop it to 1x).

Numerics: exp(2θt)-exp(2θt') is restructured as exp(2θt')*expm1(2θ dt)
(cubic Taylor expm1; dt<=1e-2 so exact to 3e-10) and 1-exp(-2θt) uses a
degree-6 Taylor/direct blend at 2θt=0.5 — both avoid catastrophic
cancellation. The 1e-8 epsilons are dropped: sqrt(t) >= 3e-2 makes them
sub-fp32-ulp, so rsqrt is used directly. bf16 rounding of noise,
coefficients, corrections and output contributes ~0.5% rms against the
2e-2 gate.
"""
import numpy as np
import ml_dtypes

import bass_rust
import concourse.bass as bass
import concourse.tile as tile
from concourse import mybir
from concourse.bass_utils import run_bass_kernel_spmd

B, T, DIM = 256, 2048, 64
THETA = 0.1
N_CORES = 8
NB = B // N_CORES      # 32 samples per core
P = 128                # partitions
L4 = 4                 # time-quad dimension
NC4 = T // (L4 * P)    # 4 time chunks of 512 steps
S = 8                  # samples packed per matmul free dim
G = NB // S            # 4 carry chains per core
H = 2                  # halves (bm / ou)
DH = DIM // H          # 32
DG = DH // 2           # 16 channel-pairs per half
FREE = S * DIM         # 512 = one PSUM bank of fp32
NF = G * S * NC4 * L4  # 512 compact coefficient columns

F32 = mybir.dt.float32
BF16 = mybir.dt.bfloat16
AF = mybir.ActivationFunctionType
OP = mybir.AluOpType


def _split_waits(nc, max_waits=1):
    """walrus in this container rejects >1 sem wait per instruction; hoist
    extras onto same-engine NoOps inserted just before the offender."""
    n = 0
    for f in nc.m.functions:
        for blk in f.blocks:
            insts = blk.instructions
            i = 0
            while i < len(insts):
                inst = insts[i]
                si = inst.sync_info
                if si is not None and len(si.on_wait) > max_waits:
                    waits = list(si.on_wait)
                    keep, rest = waits[:max_waits], waits[max_waits:]
                    nops = []
                    for j in range(0, len(rest), max_waits):
                        nop = bass_rust.InstNoOp(name=f"I-ws-{n}", ins=[], outs=[])
                        n += 1
                        nop.engine = inst.engine
                        nop.sync_info = mybir.SyncInfo(
                            on_wait=rest[j : j + max_waits], on_update=[])
                        nops.append(nop)
                    inst.sync_info = mybir.SyncInfo(
                        on_wait=keep, on_update=list(si.on_update))
                    for k, nop in enumerate(nops):
                        insts.insert(i + k, nop)
                    i += len(nops)
                i += 1
    return nc


def _strided(ap_full, offset_elems, step, count):
    """[P, count] view of a tile's free space at element offset with stride."""
    return bass.AP(
        tensor=ap_full.tensor,
        offset=ap_full.offset + offset_elems,
        ap=[list(ap_full.ap[0]), [step, count]],
    )


def _ap(ap_full, offset_elems, dims):
    """[P, *dims] view; dims = [(stride, count), ...] in free elements."""
    return bass.AP(
        tensor=ap_full.tensor,
        offset=ap_full.offset + offset_elems,
        ap=[list(ap_full.ap[0])] + [list(d) for d in dims],
    )


def _build(reps: int = 1):
    nc = bass.Bass("TRN2")
    ts_in = nc.dram_tensor("ts", [NB, T, 1], F32, kind="ExternalInput")
    nz_in = nc.dram_tensor("noise", [NB, T, DIM], BF16, kind="ExternalInput")
    out = nc.dram_tensor("out", [NB, T, DIM], BF16, kind="ExternalOutput")

    ts_flat = ts_in[:, :, 0].rearrange("s t -> (s t)")

    with nc.allow_low_precision(
            reason="bf16 pipeline by design; harness rel-err gate is 2e-2"
    ), tile.TileContext(nc) as tc:
        with (
            tc.tile_pool(name="consts", bufs=1) as consts,
            tc.tile_pool(name="cwork", bufs=1) as cwork,
            tc.tile_pool(name="nzp", bufs=8) as nzp,
            tc.tile_pool(name="rp", bufs=3) as rp,
            tc.tile_pool(name="sap", bufs=3) as sap,
            tc.tile_pool(name="op_", bufs=3) as op_,
            tc.tile_pool(name="psp", bufs=6, space="PSUM") as psp,
        ):
            # ---------------- constants ----------------
            ones_t = consts.tile([P, P], F32)
            nc.vector.memset(ones_t, 1.0)
            ones_b = consts.tile([P, P], BF16)
            nc.vector.memset(ones_b, 1.0)
            Lb = consts.tile([P, P], BF16)        # L[u, q] = 1 if u <= q
            nc.gpsimd.affine_select(
                out=Lb, in_=ones_b, pattern=[[1, P]], compare_op=OP.is_ge,
                fill=0.0, base=0, channel_multiplier=-1)
            e31 = consts.tile([32, P], F32)       # row 31 ones, else 0
            nc.gpsimd.affine_select(
                out=e31, in_=ones_t[0:32, :], pattern=[[0, P]],
                compare_op=OP.is_equal, fill=0.0, base=-31,
                channel_multiplier=1)
            ident = consts.tile([P, P], F32)      # identity for PE transpose
            nc.gpsimd.affine_select(
                out=ident, in_=ones_t, pattern=[[-1, P]],
                compare_op=OP.is_equal, fill=0.0, base=0,
                channel_multiplier=1)

            # -------- compact per-timestep coefficients --------
            # Compact layout: X[q, n], n = ((g*8+s)*4+i4)*4+lv holds
            # t = i4*512 + 4q + lv of sample 8g+s. Filled from a flat
            # full-speed ts load via PE transposes with stride-4 input APs:
            # s1[m, f] = flat[m*512+f] with m = 4*(8g+s)+i4, f = 4q+lv, so
            # transpose_lv(s1)[q, m] = s1[m, 4q+lv] lands at n = 4m+lv.
            s1 = consts.tile([P, 512], F32)       # flat[p*512 + f]
            nc.sync.dma_start(
                out=s1, in_=ts_flat.rearrange("(p f) -> p f", p=P))
            s1p = consts.tile([P, 512], F32)      # flat[p*512 + f - 1]
            nc.sync.dma_start(
                out=s1p[1:P, :],
                in_=bass.AP(tensor=ts_flat.tensor, offset=ts_flat.offset + 511,
                            ap=[[512, P - 1], [1, 512]]))
            nc.sync.dma_start(
                out=s1p[0:1, 1:512],
                in_=bass.AP(tensor=ts_flat.tensor, offset=ts_flat.offset,
                            ap=[[0, 1], [1, 511]]))
            nc.vector.memset(s1p[0:1, 0:1], 0.0)

            ts_c = consts.tile([P, G, NC4, S, L4], F32)
            tsp_c = consts.tile([P, G, NC4, S, L4], F32)
            tsf = ts_c[:, :, :, :, :].rearrange("p g i s l -> p (g i s l)")
            tspf = tsp_c[:, :, :, :, :].rearrange("p g i s l -> p (g i s l)")
            # psum column m = 4*(8g+s)+i4 lands at elem g*128 + i4*32 + s*4 + lv
            with tc.tile_pool(name="trps", bufs=2, space="PSUM") as trps:
                for src, dstf in ((s1, tsf), (s1p, tspf)):
                    for lv in range(L4):
                        pst = trps.tile([P, P], F32, tag="trp",
                                        name=f"trp{lv}")
                        nc.tensor.transpose(
                            out=pst,
                            in_=_strided(src[:, :], lv, 4, P),
                            identity=ident)
                        nc.vector.tensor_copy(
                            out=_ap(dstf, lv, [(128, G), (4, S), (32, NC4)]),
                            in_=_ap(pst[:, :], 0, [(32, G), (4, S), (1, NC4)]))
            # each sample's t=0 has predecessor time 0
            nc.vector.memset(tsp_c[0:1, :, 0:1, :, 0:1], 0.0)

            # pair-duplicated bf16 coefficient tables [q, n, h, j]
            cmul2 = consts.tile([P, NF, H, 2], BF16)
            cnorm2 = consts.tile([P, NF, H, 2], BF16)
            cmul2f = cmul2[:, :, :, :].rearrange("p n h j -> p (n h j)")
            cnorm2f = cnorm2[:, :, :, :].rearrange("p n h j -> p (n h j)")

            def pair_out(flat, h):
                return _ap(flat, 2 * h, [(4, NF), (1, 2)])

            def pair_in(flat):
                return _ap(flat, 0, [(1, NF), (0, 2)])

            t0 = cwork.tile([P, NF], F32, tag="t0")
            t1 = cwork.tile([P, NF], F32, tag="t1")
            t2 = cwork.tile([P, NF], F32, tag="t2")
            t3 = cwork.tile([P, NF], F32, tag="t3")
            t4 = cwork.tile([P, NF], F32, tag="t4")
            t0f, t1f, t3f = t0[:, :], t1[:, :], t3[:, :]

            # cm0 = sqrt(ts - tsp)   (fp32 subtraction is exact here)
            nc.vector.tensor_tensor(out=t0, in0=tsf, in1=tspf, op=OP.subtract)
            nc.scalar.activation(out=pair_out(cmul2f, 0), in_=pair_in(t0f),
                                 func=AF.Sqrt)
            # cm1 = sqrt(5 * exp(.2 tsp) * expm1(.2 (ts-tsp)))
            nc.vector.tensor_scalar_mul(out=t1, in0=t0, scalar1=0.2)     # x
            nc.vector.tensor_scalar(out=t2, in0=t1, scalar1=1.0 / 3.0,
                                    scalar2=1.0, op0=OP.mult, op1=OP.add)
            nc.vector.tensor_mul(out=t3, in0=t1, in1=t2)
            nc.vector.tensor_scalar(out=t2, in0=t3, scalar1=0.5,
                                    scalar2=1.0, op0=OP.mult, op1=OP.add)
            nc.vector.tensor_mul(out=t3, in0=t1, in1=t2)                 # expm1
            nc.scalar.activation(out=t2, in_=tspf, func=AF.Exp, scale=0.2)
            nc.vector.tensor_mul(out=t3, in0=t3, in1=t2)
            nc.scalar.activation(out=pair_out(cmul2f, 1), in_=pair_in(t3f),
                                 func=AF.Sqrt, scale=5.0)
            # cn0 = 1/sqrt(ts)  (sqrt(ts) >= 3e-2, the 1e-8 is sub-ulp)
            nc.scalar.activation(out=t2, in_=tsf, func=AF.Sqrt)
            nc.vector.reciprocal(out=pair_out(cnorm2f, 0),
                                 in_=pair_in(t2[:, :]))
            # cn1 = exp(-.1 ts) * rsqrt(5*(1-exp(-.2 ts)))
            #   1-exp(-y), y = .2 ts: Taylor (deg 6) below y=0.5 else direct
            nc.vector.tensor_scalar_mul(out=t0, in0=tsf, scalar1=0.2)    # y
            nc.scalar.activation(out=t1, in_=tsf, func=AF.Exp, scale=-0.2)
            nc.vector.tensor_scalar(out=t1, in0=t1, scalar1=-1.0,
                                    scalar2=1.0, op0=OP.mult, op1=OP.add)
            nc.vector.tensor_scalar(out=t2, in0=t0, scalar1=-1.0 / 6.0,
                                    scalar2=1.0, op0=OP.mult, op1=OP.add)
            for k in (5, 4, 3, 2):
                nc.vector.tensor_mul(out=t3, in0=t0, in1=t2)
                nc.vector.tensor_scalar(out=t2, in0=t3, scalar1=-1.0 / k,
                                        scalar2=1.0, op0=OP.mult, op1=OP.add)
            nc.vector.tensor_mul(out=t3, in0=t0, in1=t2)                 # taylor
            nc.vector.tensor_scalar(out=t4, in0=t0, scalar1=0.5, scalar2=None,
                                    op0=OP.is_lt)
            nc.vector.tensor_tensor(out=t3, in0=t3, in1=t1, op=OP.subtract)
            nc.vector.tensor_mul(out=t3, in0=t4, in1=t3)
            nc.vector.tensor_tensor(out=t3, in0=t3, in1=t1, op=OP.add)   # w2
            nc.scalar.activation(out=t3, in_=t3, func=AF.Sqrt, scale=5.0)
            nc.vector.reciprocal(out=t3, in_=t3)
            nc.scalar.activation(out=t0, in_=tsf, func=AF.Exp, scale=-0.1)
            nc.vector.tensor_tensor(
                out=pair_out(cnorm2f, 1),
                in0=pair_in(t0f), in1=pair_in(t3f), op=OP.mult)

            # ---------------- main scan ----------------
            # r-mul / norm-mul operand views per (g, i4):
            #   cmul2 elem offset for (g,i4,s,lv,h,j):
            #     n = ((g*4+i4)*8+s)*4+lv, elem = n*4 + h*2 + j
            #     -> base g*512 + i4*128; (s,lv) merge to one stride-4 dim
            ctmp = [consts.tile([32, FREE], F32, tag=f"ctmp{g}", name=f"ctmp{g}")
                    for g in range(G)]
            for _rep in range(reps):
                for i4 in range(NC4):
                    for g in range(G):
                        s8 = g * S
                        tsl = slice(i4 * L4 * P, (i4 + 1) * L4 * P)
                        cbase = g * 512 + i4 * 128
                        nt_t = nzp.tile([P, S, L4 * DIM], BF16, tag="nz")
                        nc.sync.dma_start(
                            out=nt_t,
                            in_=nz_in[s8 : s8 + S, tsl, :].rearrange(
                                "s (p l) d -> p s (l d)", l=L4))
                        # r[q, s, lv, h, dd] = nt * cmul  (one packed-pair op)
                        r = rp.tile([P, S, L4, H, DH], BF16, tag="r")
                        rf = r[:, :, :, :, :].rearrange(
                            "p s l h d -> p (s l h d)")
                        ntf = nt_t[:, :, :].rearrange("p s f -> p (s f)")
                        nc.vector.tensor_tensor(
                            out=_ap(rf, 0, [(64, S * L4), (32, H),
                                            (2, DG), (1, 2)]),
                            in0=_ap(ntf, 0, [(64, S * L4), (32, H),
                                             (2, DG), (1, 2)]),
                            in1=_ap(cmul2f, cbase,
                                    [(4, S * L4), (2, H), (0, DG), (1, 2)]),
                            op=OP.mult)
                        # S3 = sum over quads of all r_lv (+ carry)
                        ps = psp.tile([P, FREE], F32, tag="ps")
                        for lv in range(L4):
                            nc.tensor.matmul(
                                ps, lhsT=Lb, rhs=r[:, :, lv, :, :],
                                start=(lv == 0),
                                stop=(lv == 3 and i4 == 0))
                        if i4 > 0:
                            nc.tensor.matmul(ps, lhsT=e31, rhs=ctmp[g],
                                             start=False, stop=True)
                        if i4 < NC4 - 1:
                            nc.scalar.activation(out=ctmp[g], in_=ps[96:128, :],
                                                 func=AF.Copy)
                        # Sall[q, s, lv, h*dd]: lv=3 from PSUM, then descend
                        sall = sap.tile([P, S, L4, H * DH], BF16, tag="sa")
                        nc.scalar.activation(
                            out=sall[:, :, 3, :].rearrange("p s f -> p s f"),
                            in_=ps[:, :].rearrange(
                                "p (s f) -> p s f", s=S),
                            func=AF.Copy)
                        nc.gpsimd.tensor_tensor(
                            out=sall[:, :, 2, :], in0=sall[:, :, 3, :],
                            in1=r[:, :, 3, :, :].rearrange(
                                "p s h d -> p s (h d)"),
                            op=OP.subtract)
                        nc.gpsimd.tensor_tensor(
                            out=sall[:, :, 1, :], in0=sall[:, :, 2, :],
                            in1=r[:, :, 2, :, :].rearrange(
                                "p s h d -> p s (h d)"),
                            op=OP.subtract)
                        nc.vector.tensor_tensor(
                            out=sall[:, :, 0, :], in0=sall[:, :, 1, :],
                            in1=r[:, :, 1, :, :].rearrange(
                                "p s h d -> p s (h d)"),
                            op=OP.subtract)
                        # o[q, s, lv, h, dd] = Sall * cnorm (one packed-pair op)
                        o = op_.tile([P, S, L4 * DIM], BF16, tag="o")
                        of = o[:, :, :].rearrange("p s f -> p (s f)")
                        saf = sall[:, :, :, :].rearrange(
                            "p s l f -> p (s l f)")
                        nc.vector.tensor_tensor(
                            out=_ap(of, 0, [(64, S * L4), (32, H),
                                            (2, DG), (1, 2)]),
                            in0=_ap(saf, 0, [(64, S * L4), (32, H),
                                             (2, DG), (1, 2)]),
                            in1=_ap(cnorm2f, cbase,
                                    [(4, S * L4), (2, H), (0, DG), (1, 2)]),
                            op=OP.mult)
                        nc.sync.dma_start(
                            out=out[s8 : s8 + S, tsl, :].rearrange(
                                "s (p l) d -> p s (l d)", l=L4),
                            in_=o)
    _split_waits(nc)
    return nc


_NC = None


def _get_nc():
    global _NC
    if _NC is None:
        _NC = _build()
    return _NC


def kernel(ts: np.ndarray, noise: np.ndarray) -> np.ndarray:
    ts = np.ascontiguousarray(ts, dtype=np.float32)
    noise = np.ascontiguousarray(noise).astype(ml_dtypes.bfloat16)
    in_maps = [
        {"ts": ts[c * NB : (c + 1) * NB], "noise": noise[c * NB : (c + 1) * NB]}
        for c in range(N_CORES)
    ]
    res = run_bass_kernel_spmd(_get_nc(), in_maps, core_ids=list(range(N_CORES)))
    return np.concatenate(
        [r["out"] for r in res.results], axis=0).astype(np.float32)


# revision 11
# speedup vs baseline: 1.8163x; 1.3271x over previous
"""Brownian/OU bridge sampler kernel for Trainium2 (8 NeuronCores), bf16 edition.

Problem (per batch element b, time series of length T, DIM=64 channels):
  first 32 channels:  bm = cumsum_t(sqrt(dt)*noise) / (sqrt(t)+1e-8)
  last 32 channels:   ou = e^{-theta t} * cumsum_t(sqrt(e^{2 theta t}-e^{2 theta t'})
                           * sigma/sqrt(2 theta) * noise)
                           / (sigma*sqrt((1-e^{-2 theta t})/(2 theta))+1e-8)

Strategy: pure data parallel over batch (32 samples per core). The rel-err
gate (2e-2) leaves room for bf16 data movement, which halves HBM traffic vs
the f32 pipeline: noise is staged to the device as bf16 and the output is
returned as bf16 (widened to f32 on the host). To keep 512-byte DMA
descriptors (the SDMA line-rate knee) at 2 bytes/elem, each 512-timestep
chunk uses a time-QUAD layout: partition q holds t = 4q+lv, lv=0..3. The
512-step cumsum is 4 accumulating bf16 matmuls against a triangular-ones
stationary (S3 = sum of all r_lv up to partition q), plus an f32 carry
matmul that broadcasts PSUM row 127 of the previous chunk from a 32-row
stash. S2/S1/S0 descend from S3 by subtracting r3/r2/r1 (spread across
Pool and DVE); a single fused DVE multiply applies the normalization.

Per-timestep coefficients are computed once in f32 on a compact [128, 512]
layout (flat full-speed ts load + 8 PE-transpose redistributions with
stride-4 access patterns), then written as PAIR-DUPLICATED bf16 tables
[q, n, 2] holding (c, c). Broadcasting a coefficient across the 32
channels of a half is done with access patterns whose innermost dim is the
packed pair (stride 1, count 2) and whose channel dim is stride 0 — every
operand then presents a packed 2-byte innermost axis, which keeps the DVE
in its 2x half-word mode (a stride-0 innermost axis would drop it to 1x).

Numerics: exp(2θt)-exp(2θt') is restructured as exp(2θt')*expm1(2θ dt)
(cubic Taylor expm1; dt<=1e-2 so exact to 3e-10) and 1-exp(-2θt) uses a
degree-6 Taylor/direct blend at 2θt=0.5 — both avoid catastrophic
cancellation. The 1e-8 epsilons are dropped: sqrt(t) >= 3e-2 makes them
sub-fp32-ulp, so rsqrt is used directly. bf16 rounding of noise,
coefficients, corrections and output contributes ~0.5% rms against the
2e-2 gate.
"""
import numpy as np
import ml_dtypes

import bass_rust
import concourse.bass as bass
import concourse.tile as tile
from concourse import mybir
from concourse.bass_utils import run_bass_kernel_spmd

B, T, DIM = 256, 2048, 64
THETA = 0.1
N_CORES = 8
NB = B // N_CORES      # 32 samples per core
P = 128                # partitions
L4 = 4                 # time-quad dimension
NC4 = T // (L4 * P)    # 4 time chunks of 512 steps
S = 8                  # samples packed per matmul free dim
G = NB // S            # 4 carry chains per core
H = 2                  # halves (bm / ou)
DH = DIM // H          # 32
DG = DH // 2           # 16 channel-pairs per half
FREE = S * DIM         # 512 = one PSUM bank of fp32
NF = G * S * NC4 * L4  # 512 compact coefficient columns

F32 = mybir.dt.float32
F32R = mybir.dt.float32r
BF16 = mybir.dt.bfloat16
AF = mybir.ActivationFunctionType
OP = mybir.AluOpType


def _split_waits(nc, max_waits=1):
    """walrus in this container rejects >1 sem wait per instruction; hoist
    extras onto same-engine NoOps inserted just before the offender."""
    n = 0
    for f in nc.m.functions:
        for blk in f.blocks:
            insts = blk.instructions
            i = 0
            while i < len(insts):
                inst = insts[i]
                si = inst.sync_info
                if si is not None and len(si.on_wait) > max_waits:
                    waits = list(si.on_wait)
                    keep, rest = waits[:max_waits], waits[max_waits:]
                    nops = []
                    for j in range(0, len(rest), max_waits):
                        nop = bass_rust.InstNoOp(name=f"I-ws-{n}", ins=[], outs=[])
                        n += 1
                        nop.engine = inst.engine
                        nop.sync_info = mybir.SyncInfo(
                            on_wait=rest[j : j + max_waits], on_update=[])
                        nops.append(nop)
                    inst.sync_info = mybir.SyncInfo(
                        on_wait=keep, on_update=list(si.on_update))
                    for k, nop in enumerate(nops):
                        insts.insert(i + k, nop)
                    i += len(nops)
                i += 1
    return nc


def _strided(ap_full, offset_elems, step, count):
    """[P, count] view of a tile's free space at element offset with stride."""
    return bass.AP(
        tensor=ap_full.tensor,
        offset=ap_full.offset + offset_elems,
        ap=[list(ap_full.ap[0]), [step, count]],
    )


def _ap(ap_full, offset_elems, dims):
    """[P, *dims] view; dims = [(stride, count), ...] in free elements."""
    return bass.AP(
        tensor=ap_full.tensor,
        offset=ap_full.offset + offset_elems,
        ap=[list(ap_full.ap[0])] + [list(d) for d in dims],
    )


def _build(reps: int = 1):
    nc = bass.Bass("TRN2")
    ts_in = nc.dram_tensor("ts", [NB, T, 1], F32, kind="ExternalInput")
    nz_in = nc.dram_tensor("noise", [NB, T, DIM], BF16, kind="ExternalInput")
    out = nc.dram_tensor("out", [NB, T, DIM], BF16, kind="ExternalOutput")

    ts_flat = ts_in[:, :, 0].rearrange("s t -> (s t)")

    with nc.allow_low_precision(
            reason="bf16 pipeline by design; harness rel-err gate is 2e-2"
    ), tile.TileContext(nc) as tc:
        with (
            tc.tile_pool(name="consts", bufs=1) as consts,
            tc.tile_pool(name="cwork", bufs=1) as cwork,
            tc.tile_pool(name="nzp", bufs=3) as nzp,
            tc.tile_pool(name="rp", bufs=5) as rp,
            tc.tile_pool(name="sap", bufs=5) as sap,
            tc.tile_pool(name="op_", bufs=5) as op_,
            tc.tile_pool(name="psp", bufs=6, space="PSUM") as psp,
        ):
            # ---------------- constants ----------------
            ones_t = consts.tile([P, P], F32)
            nc.vector.memset(ones_t, 1.0)
            ones_b = consts.tile([P, P], BF16)
            nc.vector.memset(ones_b, 1.0)
            Lb = consts.tile([P, P], BF16)        # L[u, q] = 1 if u <= q
            nc.gpsimd.affine_select(
                out=Lb, in_=ones_b, pattern=[[1, P]], compare_op=OP.is_ge,
                fill=0.0, base=0, channel_multiplier=-1)
            e31 = consts.tile([32, P], F32R)      # row 31 ones, else 0
            nc.gpsimd.affine_select(
                out=e31, in_=ones_t[0:32, :], pattern=[[0, P]],
                compare_op=OP.is_equal, fill=0.0, base=-31,
                channel_multiplier=1)
            ident = consts.tile([P, P], F32)      # identity for PE transpose
            nc.gpsimd.affine_select(
                out=ident, in_=ones_t, pattern=[[-1, P]],
                compare_op=OP.is_equal, fill=0.0, base=0,
                channel_multiplier=1)

            # -------- compact per-timestep coefficients --------
            # Compact layout: X[q, n], n = ((g*8+s)*4+i4)*4+lv holds
            # t = i4*512 + 4q + lv of sample 8g+s. Filled from a flat
            # full-speed ts load via PE transposes with stride-4 input APs:
            # s1[m, f] = flat[m*512+f] with m = 4*(8g+s)+i4, f = 4q+lv, so
            # transpose_lv(s1)[q, m] = s1[m, 4q+lv] lands at n = 4m+lv.
            s1 = consts.tile([P, 512], F32)       # flat[p*512 + f]
            nc.sync.dma_start(
                out=s1, in_=ts_flat.rearrange("(p f) -> p f", p=P))
            s1p = consts.tile([P, 512], F32)      # flat[p*512 + f - 1]
            nc.sync.dma_start(
                out=s1p[1:P, :],
                in_=bass.AP(tensor=ts_flat.tensor, offset=ts_flat.offset + 511,
                            ap=[[512, P - 1], [1, 512]]))
            nc.sync.dma_start(
                out=s1p[0:1, 1:512],
                in_=bass.AP(tensor=ts_flat.tensor, offset=ts_flat.offset,
                            ap=[[0, 1], [1, 511]]))
            nc.vector.memset(s1p[0:1, 0:1], 0.0)

            ts_c = consts.tile([P, G, NC4, S, L4], F32)
            tsp_c = consts.tile([P, G, NC4, S, L4], F32)
            tsf = ts_c[:, :, :, :, :].rearrange("p g i s l -> p (g i s l)")
            tspf = tsp_c[:, :, :, :, :].rearrange("p g i s l -> p (g i s l)")
            # psum column m = 4*(8g+s)+i4 lands at elem g*128 + i4*32 + s*4 + lv
            with tc.tile_pool(name="trps", bufs=2, space="PSUM") as trps:
                for src, dstf in ((s1, tsf), (s1p, tspf)):
                    for lv in range(L4):
                        pst = trps.tile([P, P], F32, tag="trp",
                                        name=f"trp{lv}")
                        nc.tensor.transpose(
                            out=pst,
                            in_=_strided(src[:, :], lv, 4, P),
                            identity=ident)
                        nc.vector.tensor_copy(
                            out=_ap(dstf, lv, [(128, G), (4, S), (32, NC4)]),
                            in_=_ap(pst[:, :], 0, [(32, G), (4, S), (1, NC4)]))
            # each sample's t=0 has predecessor time 0
            nc.vector.memset(tsp_c[0:1, :, 0:1, :, 0:1], 0.0)

            # pair-duplicated bf16 coefficient tables [q, n, h, j]
            cmul2 = consts.tile([P, NF, H, 2], BF16)
            cnorm2 = consts.tile([P, NF, H, 2], BF16)
            cmul2f = cmul2[:, :, :, :].rearrange("p n h j -> p (n h j)")
            cnorm2f = cnorm2[:, :, :, :].rearrange("p n h j -> p (n h j)")

            def pair_out(flat, h):
                return _ap(flat, 2 * h, [(4, NF), (1, 2)])

            def pair_in(flat):
                return _ap(flat, 0, [(1, NF), (0, 2)])

            t0 = cwork.tile([P, NF], F32, tag="t0")
            t1 = cwork.tile([P, NF], F32, tag="t1")
            t2 = cwork.tile([P, NF], F32, tag="t2")
            t3 = cwork.tile([P, NF], F32, tag="t3")
            t4 = cwork.tile([P, NF], F32, tag="t4")
            t0f, t1f, t3f = t0[:, :], t1[:, :], t3[:, :]

            # cm0 = sqrt(ts - tsp)   (fp32 subtraction is exact here)
            nc.vector.tensor_tensor(out=t0, in0=tsf, in1=tspf, op=OP.subtract)
            nc.scalar.activation(out=pair_out(cmul2f, 0), in_=pair_in(t0f),
                                 func=AF.Sqrt)
            # cn1 = exp(-.1 ts) / sqrt(5*(1-exp(-.2 ts)))
            #     = 1 / (exp(.1 ts) * sqrt(5*(1 - u^2))),  u = exp(-.1 ts).
            # Direct 1-u^2 costs ~1e-5/(0.2 t) relative at the smallest t;
            # only the first timestep sees >1%, invisible in the 2e-2 norm
            # gate, so the Taylor blend of the f32 pipeline is dropped.
            nc.scalar.activation(out=t1, in_=tsf, func=AF.Exp, scale=-0.1)
            nc.gpsimd.tensor_mul(out=t2, in0=t1, in1=t1)                 # u^2
            nc.vector.tensor_scalar(out=t2, in0=t2, scalar1=-1.0,
                                    scalar2=1.0, op0=OP.mult, op1=OP.add)
            nc.scalar.activation(out=t2, in_=t2, func=AF.Sqrt, scale=5.0)
            nc.scalar.activation(out=t4, in_=tsf, func=AF.Exp, scale=0.1)
            nc.gpsimd.tensor_mul(out=t4, in0=t4, in1=t2)
            nc.vector.reciprocal(out=pair_out(cnorm2f, 1),
                                 in_=pair_in(t4[:, :]))
            # cn0 = 1/sqrt(ts)  (sqrt(ts) >= 3e-2, the 1e-8 is sub-ulp)
            nc.scalar.activation(out=t2, in_=tsf, func=AF.Sqrt)
            nc.vector.reciprocal(out=pair_out(cnorm2f, 0),
                                 in_=pair_in(t2[:, :]))
            # cm1 = sqrt(5 * exp(.2 tsp) * expm1(.2 dt)), cubic-Taylor expm1
            # (kept: a direct exp difference would see the full ~1e-5 ACT
            # error against expm1 ~ 2e-4..2e-3 on every element)
            nc.vector.tensor_scalar_mul(out=t1, in0=t0, scalar1=0.2)     # x
            nc.vector.tensor_scalar(out=t2, in0=t1, scalar1=1.0 / 3.0,
                                    scalar2=1.0, op0=OP.mult, op1=OP.add)
            nc.gpsimd.tensor_mul(out=t3, in0=t1, in1=t2)
            nc.vector.tensor_scalar(out=t2, in0=t3, scalar1=0.5,
                                    scalar2=1.0, op0=OP.mult, op1=OP.add)
            nc.gpsimd.tensor_mul(out=t3, in0=t1, in1=t2)                 # expm1
            nc.scalar.activation(out=t2, in_=tspf, func=AF.Exp, scale=0.2)
            nc.gpsimd.tensor_mul(out=t3, in0=t3, in1=t2)
            nc.scalar.activation(out=pair_out(cmul2f, 1), in_=pair_in(t3f),
                                 func=AF.Sqrt, scale=5.0)

            # ---------------- main scan ----------------
            # r-mul / norm-mul operand views per (g, i4):
            #   cmul2 elem offset for (g,i4,s,lv,h,j):
            #     n = ((g*4+i4)*8+s)*4+lv, elem = n*4 + h*2 + j
            #     -> base g*512 + i4*128; (s,lv) merge to one stride-4 dim
            ctmp = [consts.tile([32, FREE], F32R, tag=f"ctmp{g}",
                                name=f"ctmp{g}")
                    for g in range(G)]
            for _rep in range(reps):
                for i4 in range(NC4):
                    tsl = slice(i4 * L4 * P, (i4 + 1) * L4 * P)
                    nt_t = nzp.tile([P, NB, L4 * DIM], BF16, tag="nz")
                    nc.sync.dma_start(
                        out=nt_t,
                        in_=nz_in[:, tsl, :].rearrange(
                            "s (p l) d -> p s (l d)", l=L4))
                    for g in range(G):
                        s8 = g * S
                        cbase = g * 512 + i4 * 128
                        # r[q, s, lv, h, dd] = nt * cmul  (one packed-pair op)
                        r = rp.tile([P, S, L4, H, DH], BF16, tag="r")
                        rf = r[:, :, :, :, :].rearrange(
                            "p s l h d -> p (s l h d)")
                        ntf = nt_t[:, s8 : s8 + S, :].rearrange(
                            "p s f -> p (s f)")
                        nc.vector.tensor_tensor(
                            out=_ap(rf, 0, [(64, S * L4), (32, H),
                                            (2, DG), (1, 2)]),
                            in0=_ap(ntf, 0, [(64, S * L4), (32, H),
                                             (2, DG), (1, 2)]),
                            in1=_ap(cmul2f, cbase,
                                    [(4, S * L4), (2, H), (0, DG), (1, 2)]),
                            op=OP.mult)
                        # S3 = sum over quads of all r_lv (+ carry)
                        ps = psp.tile([P, FREE], F32, tag="ps")
                        if i4 > 0:
                            nc.tensor.matmul(ps, lhsT=e31, rhs=ctmp[g],
                                             start=True, stop=False)
                        for lv in range(L4):
                            nc.tensor.matmul(
                                ps, lhsT=Lb, rhs=r[:, :, lv, :, :],
                                start=(lv == 0 and i4 == 0),
                                stop=(lv == 3))
                        if i4 < NC4 - 1:
                            nc.scalar.activation(out=ctmp[g], in_=ps[96:128, :],
                                                 func=AF.Copy)
                        # Sall[q, s, lv, h*dd]: lv=3 from PSUM, then descend
                        sall = sap.tile([P, S, L4, H * DH], BF16, tag="sa")
                        nc.scalar.activation(
                            out=sall[:, :, 3, :].rearrange("p s f -> p s f"),
                            in_=ps[:, :].rearrange(
                                "p (s f) -> p s f", s=S),
                            func=AF.Copy)
                        nc.gpsimd.tensor_tensor(
                            out=sall[:, :, 2, :], in0=sall[:, :, 3, :],
                            in1=r[:, :, 3, :, :].rearrange(
                                "p s h d -> p s (h d)"),
                            op=OP.subtract)
                        nc.gpsimd.tensor_tensor(
                            out=sall[:, :, 1, :], in0=sall[:, :, 2, :],
                            in1=r[:, :, 2, :, :].rearrange(
                                "p s h d -> p s (h d)"),
                            op=OP.subtract)
                        nc.vector.tensor_tensor(
                            out=sall[:, :, 0, :], in0=sall[:, :, 1, :],
                            in1=r[:, :, 1, :, :].rearrange(
                                "p s h d -> p s (h d)"),
                            op=OP.subtract)
                        # o[q, s, lv, h, dd] = Sall * cnorm (one packed-pair op)
                        o = op_.tile([P, S, L4 * DIM], BF16, tag="o")
                        of = o[:, :, :].rearrange("p s f -> p (s f)")
                        saf = sall[:, :, :, :].rearrange(
                            "p s l f -> p (s l f)")
                        nc.vector.tensor_tensor(
                            out=_ap(of, 0, [(64, S * L4), (32, H),
                                            (2, DG), (1, 2)]),
                            in0=_ap(saf, 0, [(64, S * L4), (32, H),
                                             (2, DG), (1, 2)]),
                            in1=_ap(cnorm2f, cbase,
                                    [(4, S * L4), (2, H), (0, DG), (1, 2)]),
                            op=OP.mult)
                        nc.scalar.dma_start(
                            out=out[s8 : s8 + S, tsl, :].rearrange(
                                "s (p l) d -> p s (l d)", l=L4),
                            in_=o)
    _split_waits(nc)
    return nc


_NC = None


def _get_nc():
    global _NC
    if _NC is None:
        _NC = _build()
    return _NC


def kernel(ts: np.ndarray, noise: np.ndarray) -> np.ndarray:
    ts = np.ascontiguousarray(ts, dtype=np.float32)
    noise = np.ascontiguousarray(noise).astype(ml_dtypes.bfloat16)
    in_maps = [
        {"ts": ts[c * NB : (c + 1) * NB], "noise": noise[c * NB : (c + 1) * NB]}
        for c in range(N_CORES)
    ]
    res = run_bass_kernel_spmd(_get_nc(), in_maps, core_ids=list(range(N_CORES)))
    return np.concatenate(
        [r["out"] for r in res.results], axis=0).astype(np.float32)
